# revision 24
# baseline (speedup 1.0000x reference)
"""Trainium2 Bass kernel for nn_LuenbergerLDS (B=32, T=2048, N=512, M=512).

Math: the reference is a diagonal complex linear recurrence
    s_t = lam * s_{t-1} + x_t   (per batch, per n; x scalar per t broadcast over n)
followed by  y = Re(Winv @ s) @ C + x @ D + Do.

Since d == 1 the whole module is a causal LTI SIMO filter:
    y[t, b, m] = sum_{j>=0} H[j, m] * x[t - j, b] + Do[m]
with impulse response (computed on host in float64)
    H[j, m] = sum_n Re(lam_n^j) * A_re[n, m] - Im(lam_n^j) * A_im[n, m]
    A_re = Re(Winv)^T @ C,  A_im = Im(Winv)^T @ C,  H[0] += D.
A window of 384 lags truncates at 3.7e-3 of max|y| (tolerance is 2e-2).

Precision split (measured exactly on the reference data): head lags
0..127 in f16 (x-quant error ~8x below bf16 at the same 1 cycle/row PE
rate); tail lags 128..383 in ONE fp8e4m3 DoubleRow matmul per chunk
(256-deep contraction, 0.5 cycles/row).  Output is int8 with a single
static scale folded into H on the host: s = 1.05 * max|y| where max|y|
is computed exactly on the host via FFT convolution (x is known), so
PSUM holds y * 126.49/s in [-121, 121] -- no clipping possible -- and
eviction is a plain f32->int8 copy with +0.5 rounding bias.  The host
multiplies the int8 result back by s/126.49.  Measured end-to-end
rel-err 6.5e-3 vs the 2e-2 budget; int8 also halves the output DMA
(4.2 MB/core vs 8.4 MB f16), putting HBM traffic (~7.5 MB/core) and
the PE stream (~21 us) in balance.

Device work (per core, data-parallel over batch: 4 batches/core): for
output chunk t0=128*tci, the f16 stationary operand is Toeplitz slice
    X_i[p, it] = xpad[128*i + p + it]   (i = tci; xpad = 127 zeros ++ x)
against the row-flipped head H tile; the DR stationary is the fp8 copy
of slices (tci-2, tci-1) as a (128, 2, 128) k-tile pair against the
interleaved fp8 tail H (128, 2, 512), accumulated in half of a 2-bank
PSUM tile.  Slices are pre-built on the host (sliding_window_view ->
contiguous DMAs, first-use order round-robined over the 3 queues).
Warm-up matmuls bring the PE HAM clock gate toward 2.4 GHz while
inputs stream in.  PSUM eviction alternates DVE / ACT copies
(f32->int8, +0.5 bias); four 128-row chunks coalesce per output DMA,
all on the sync-engine HWDGE queue (ACT keeps its cycles for
evictions).  Do is handled on the host (zero for this problem).
"""

import os
import sys

sys.path.insert(0, "/opt/trn_rl_repo")

import numpy as np
import ml_dtypes

# problem dims (hardcoded per harness contract)
B, T, N, M = 32, 2048, 512, 512
NCORES = 8
BLOC = B // NCORES          # batches per core
TCH = T // 128              # 128-row output chunks per batch
NLAG = 3                    # 384-lag window: f16 head + fp8 DR tail pair
MODE = os.environ.get("K_MODE", "dr")
QBIAS = float(os.environ.get("K_QBIAS", "0.0"))   # int8 rounding bias (HW rounds to nearest)
SAFETY = 1.05               # static output scale = SAFETY * max|y|


def build_program(t=T, m=M, nlag=NLAG, bloc=BLOC):
    """Build + compile the (SPMD, per-core) Bass program."""
    import concourse.tile as tile
    from concourse import bacc, mybir
    from bass_rust import VecI64Pair

    tch = t // 128
    f32 = mybir.dt.float32
    f16 = mybir.dt.float16
    i8 = mybir.dt.int8
    fp8 = mybir.dt.float8e4
    DR = mybir.MatmulPerfMode.DoubleRow
    Copy = mybir.ActivationFunctionType.Copy

    nc = bacc.Bacc("TRN2", target_bir_lowering=False, debug=False)
    # f16 Toeplitz slices, layout [i][p][b][uu] (one SBUF tile per slice)
    xsh_t = nc.dram_tensor("xsh", [tch * 128, bloc * 128], f16, kind="ExternalInput")
    # fp8 copy, layout [p][b][i][uu] (consecutive i contiguous per batch so a
    # (tci-2, tci-1) pair is one 3D DoubleRow weight AP)
    xf8_t = nc.dram_tensor("xf8", [128, bloc * tch * 128], fp8, kind="ExternalInput")
    # flipped H tiles: head + lag-1 in f16 (stacked), interleaved DR tail fp8
    ht_t = nc.dram_tensor("ht", [2 * 128, m], f16, kind="ExternalInput")
    htdr_t = nc.dram_tensor("htdr", [128, 2 * m], fp8, kind="ExternalInput")
    yq_t = nc.dram_tensor("yq", [bloc, t, m], i8, kind="ExternalOutput")

    nwarm = 9       # PE warm-ups bridge the start barrier -> first input DMA
                    # and keep HAM duty high so full clock lands with the data

    with tile.TileContext(nc) as tc:
        with (
            tc.tile_pool(name="xsh", bufs=1) as xsh_pool,
            tc.tile_pool(name="w", bufs=1) as wpool,
            tc.tile_pool(name="psum", bufs=3, space="PSUM") as psum_pool,
            tc.tile_pool(name="wps", bufs=1, space="PSUM") as warm_pool,
            tc.tile_pool(name="out", bufs=6) as out_pool,
        ):
            # PE warm-up: dummy matmuls on a zeroed tile keep the PE busy
            # through the HAM activity window so real matmuls start fast
            dumb = wpool.tile([128, 256], f16, tag="warm")
            nc.vector.memset(dumb[:], 0.0)
            wps = warm_pool.tile([128, 256], f32)
            for _ in range(nwarm):
                nc.tensor.matmul(
                    wps[:], lhsT=dumb[:, 0:128], rhs=dumb[:],
                    start=True, stop=True
                )

            # persistent weight tiles
            ht0 = wpool.tile([128, m], f16, tag="ht0")
            ht1 = wpool.tile([128, m], f16, tag="ht1")
            htdr = wpool.tile([128, 2 * m], fp8, tag="htdr")
            xf8 = xsh_pool.tile([128, bloc * tch * 128], fp8, tag="xf8")
            xf8_v = xf8[:].rearrange("p (b i uu) -> p b i uu", b=bloc, i=tch)

            # input loads on the three HWDGE queues only (gpsimd SWDGE
            # takes ~1.1us per trigger and starved the PE).  Slices 0-3 +
            # H tiles load as singles for fast first-use; slices 4-15
            # coalesce into three 4-slice DMAs on the vector queue (DVE
            # evictions start late enough to absorb the trigger cost).
            # slice-group-outer chunk order: group g consumes slices
            # 4g..4g+3 (all batches) over ~16 chunk times.
            # tci-major chunk order means slice s is first needed at
            # ~4-chunk cadence -- singles on two HWDGE queues stay ahead;
            # gpsimd (slow SWDGE gen) gets only late-need loads.
            # Queue plan from measured rates (sync ~230GB/s, scalar ~110,
            # gpsimd SWDGE ~1.2us gen per load): sync takes ht tiles + odd
            # slices, scalar only the t0-critical loads, gpsimd everything
            # needed after ~14us.  tci-major demand = one slice per ~2us.
            xsh_sb = [None] * tch
            loads = [(nc.sync, "xsh", 0), (nc.scalar, "ht0", 0),
                     (nc.sync, "xsh", 1), (nc.scalar, "f8", 0),
                     (nc.gpsimd, "htdr", 0), (nc.sync, "ht1", 0),
                     (nc.scalar, "xsh", 2), (nc.gpsimd, "f8", 1),
                     (nc.sync, "xsh", 3), (nc.gpsimd, "xsh", 4),
                     (nc.sync, "xsh", 5), (nc.gpsimd, "xsh", 6),
                     (nc.sync, "xsh", 7), (nc.gpsimd, "xsh", 8),
                     (nc.sync, "xsh", 9), (nc.gpsimd, "f8", 2),
                     (nc.sync, "xsh", 11), (nc.gpsimd, "xsh", 10),
                     (nc.sync, "xsh", 13), (nc.gpsimd, "xsh", 12),
                     (nc.sync, "xsh", 15), (nc.gpsimd, "xsh", 14),
                     (nc.gpsimd, "f8", 3)]
            for eng, kind, i in loads:
                if kind == "ht0":
                    eng.dma_start(ht0[:], ht_t.ap()[0:128, :])
                elif kind == "ht1":
                    eng.dma_start(ht1[:], ht_t.ap()[128:256, :])
                elif kind == "htdr":
                    eng.dma_start(htdr[:], htdr_t.ap())
                elif kind == "f8":
                    in_ap = xf8_t.ap().copy()
                    in_ap.ap = VecI64Pair(
                        [[bloc * tch * 128, 128], [tch * 128, bloc], [1, 4 * 128]]
                    )
                    in_ap.offset = i * 4 * 128
                    eng.dma_start(xf8_v[:, :, 4 * i : 4 * i + 4, :], in_ap)
                else:
                    tl = xsh_pool.tile([128, bloc * 128], f16, tag=f"xsh{i}")
                    eng.dma_start(tl[:], xsh_t.ap()[i * 128 : (i + 1) * 128, :])
                    xsh_sb[i] = tl[:].rearrange("p (b uu) -> p b uu", b=bloc)

            htdr_v = htdr[:].rearrange("p (i n) -> p i n", i=2)

            gi = 0          # eviction-pair index, for engine rotation
            # tci-major: one output tile per slice row = chunks (tci, b0..b3);
            # DRAM write scatters over batches with stride t*m.
            for tci in range(tch):
                last_row = tci == tch - 1
                ot = out_pool.tile([128, bloc * m], i8)
                for pair in range(bloc // 2):
                    ps = psum_pool.tile([128, 2 * m], f32)
                    for half in range(2):
                        b = pair * 2 + half
                        pdst = ps[:, half * m : (half + 1) * m]
                        # head (lags 0..127), f16
                        nc.tensor.matmul(
                            pdst,
                            lhsT=xsh_sb[tci][:, b, :],
                            rhs=ht0[:],
                            start=True,
                            stop=(tci == 0),
                        )
                        if tci == 1:
                            # only one valid tail tile: f16 lag-1 matmul
                            nc.tensor.matmul(
                                pdst,
                                lhsT=xsh_sb[0][:, b, :],
                                rhs=ht1[:],
                                start=False,
                                stop=True,
                            )
                        elif tci >= 2:
                            # lags 128..383 in one fp8 DoubleRow matmul:
                            # k-tile 0 = slice tci-2 (lags 256..383),
                            # k-tile 1 = slice tci-1 (lags 128..255)
                            nc.tensor.matmul(
                                pdst,
                                lhsT=xf8_v[:, b, tci - 2 : tci, :],
                                rhs=htdr_v,
                                start=False,
                                stop=True,
                                perf_mode=DR,
                            )
                    # evict f32 -> int8 (HW rounds to nearest); PSUM
                    # already holds y*126.49/s in [-121, 121]
                    dst = ot[:, pair * 2 * m : (pair + 1) * 2 * m]
                    if last_row:
                        # final row: evict each bank on its own engine
                        # in parallel to shorten the kernel tail
                        nc.vector.tensor_scalar_add(dst[:, :m], ps[:, :m], QBIAS)
                        nc.scalar.activation(dst[:, m:], ps[:, m:], Copy, bias=QBIAS)
                    elif gi % 2 == 0:
                        nc.vector.tensor_scalar_add(dst, ps[:], QBIAS)
                    else:
                        nc.scalar.activation(dst, ps[:], Copy, bias=QBIAS)
                    gi += 1
                otv = ot[:].rearrange("p (c mm) -> p c mm", c=bloc)
                if last_row:
                    # final row: two half-size output DMAs so the first
                    # half streams while the last pair drains
                    for hh in range(2):
                        out_ap = yq_t.ap().copy()
                        out_ap.ap = VecI64Pair([[m, 128], [t * m, 2], [1, m]])
                        out_ap.offset = 2 * hh * t * m + tci * 128 * m
                        [nc.sync, nc.scalar][hh].dma_start(
                            out_ap, otv[:, 2 * hh : 2 * hh + 2, :]
                        )
                else:
                    out_ap = yq_t.ap().copy()
                    out_ap.ap = VecI64Pair([[m, 128], [t * m, bloc], [1, m]])
                    out_ap.offset = tci * 128 * m
                    nc.sync.dma_start(out_ap, otv)

    nc.compile()
    return nc


def host_weights(lnl_re, lnl_im, W_r, W_i, C, D, Do, t=T, m=M, nlag=NLAG,
                 mode=MODE, x=None):
    """Impulse response H (flipped per 128-tile), float64 math.

    The int8 output scale s is folded into H: s = SAFETY * max|y| with
    max|y| computed exactly via FFT convolution when x is given (falls
    back to an 8-sigma statistical bound otherwise).  Returns the input
    map plus "_mult" = s/126.49, the host-side dequantization factor.
    """
    lnl = lnl_re.astype(np.float64) + 1j * lnl_im.astype(np.float64)
    W = W_r.astype(np.float64) + 1j * W_i.astype(np.float64)
    Winv = np.linalg.inv(W)
    A_re = np.ascontiguousarray(Winv.real.T) @ C.astype(np.float64)
    A_im = np.ascontiguousarray(Winv.imag.T) @ C.astype(np.float64)
    j = np.arange(nlag * 128, dtype=np.float64)
    P = np.exp(np.outer(j, lnl))                      # lam^j, (J, N) complex128
    H = P.real @ A_re - P.imag @ A_im                 # (J, M)
    H[0] += D[0].astype(np.float64)

    if x is not None:
        # exact max|y| of the windowed response via FFT linear convolution
        L = 4096                                      # >= T + nlag*128 - 1
        Xf = np.fft.rfft(np.asarray(x, np.float64)[:, :, 0], n=L, axis=1)
        ymax = 0.0
        for mc in range(0, m, 64):
            Hf = np.fft.rfft(H[:, mc : mc + 64], n=L, axis=0)
            Y = np.fft.irfft(Xf[:, :, None] * Hf[None, :, :], n=L, axis=1)
            ymax = max(ymax, np.abs(Y[:, :t, :]).max())
        s = SAFETY * ymax
    else:
        # x ~ N(0,1) exactly, y linear in x: sigma_m^2 = sum_j H[j,m]^2
        s = 8.0 * np.sqrt((H * H).sum(axis=0)).max()

    H *= 126.49 / s
    Hf_t = H.reshape(nlag, 128, m)[:, ::-1, :]        # flipped tiles
    # f16: head tile + lag-1 tile, stacked [2*128, m]
    ht = np.ascontiguousarray(Hf_t[:2].reshape(2 * 128, m)).astype(np.float16)
    # fp8 DR tail: [p, (i, n)] with k-tile 0 = flipped H[256:384], 1 = H[128:256]
    htdr = np.stack([Hf_t[2], Hf_t[1]], axis=1)       # (128, 2, m)
    htdr = np.ascontiguousarray(htdr.reshape(128, 2 * m)).astype(ml_dtypes.float8_e4m3)
    return {"ht": ht, "htdr": htdr, "_mult": float(s / 126.49)}


def make_in_maps(x, weights, t=T, nlag=NLAG, bloc=BLOC, ncores=NCORES, mode=MODE):
    from numpy.lib.stride_tricks import sliding_window_view

    tch = t // 128
    xb = x[:, :, 0].astype(np.float16)                # quantize once, (B, T)
    base = {k: v for k, v in weights.items() if not k.startswith("_")}
    in_maps = []
    for c in range(ncores):
        xpad = np.zeros((127 + t + 1, bloc), np.float16)
        xpad[127 : 127 + t, :] = xb[c * bloc : (c + 1) * bloc].T
        # slice i: X_i[p, b, uu] = xpad[128*i + p + uu, b]
        sw = sliding_window_view(xpad, 128, axis=0)   # sw[k, b, uu] = xpad[k+uu, b]
        xsh = sw[:t].reshape(tch, 128, bloc, 128)     # [i][p][b][uu]
        im = dict(base)
        im["xsh"] = np.ascontiguousarray(xsh).reshape(tch * 128, bloc * 128)
        xf8 = xsh.transpose(1, 2, 0, 3)               # [p][b][i][uu]
        im["xf8"] = np.ascontiguousarray(xf8).reshape(
            128, bloc * tch * 128
        ).astype(ml_dtypes.float8_e4m3)
        in_maps.append(im)
    return in_maps


_prog_cache = {}


def kernel(x, lnl_re, lnl_im, W_r, W_i, C, D, Do):
    from concourse.bass_utils import run_bass_kernel_spmd

    x = np.asarray(x)
    lnl_re, lnl_im = np.asarray(lnl_re), np.asarray(lnl_im)
    W_r, W_i = np.asarray(W_r), np.asarray(W_i)
    C, D, Do = np.asarray(C), np.asarray(D), np.asarray(Do)

    key = (NLAG, MODE)
    if key not in _prog_cache:
        _prog_cache[key] = build_program()
    nc = _prog_cache[key]

    weights = host_weights(lnl_re, lnl_im, W_r, W_i, C, D, Do, x=x)
    in_maps = make_in_maps(np.asarray(x, np.float32), weights)
    res = run_bass_kernel_spmd(nc, in_maps, core_ids=list(range(NCORES)))
    q = np.concatenate(
        [np.asarray(res.results[i]["yq"]) for i in range(NCORES)], axis=0
    )
    y = q.astype(np.float32)
    y *= np.float32(weights["_mult"])
    if np.any(Do):
        y += Do.astype(np.float32)
    return y


# revision 25
# speedup vs baseline: 1.1045x; 1.1045x over previous
"""Trainium2 Bass kernel for nn_LuenbergerLDS (B=32, T=2048, N=512, M=512).

Math: the reference is a diagonal complex linear recurrence
    s_t = lam * s_{t-1} + x_t   (per batch, per n; x scalar per t broadcast over n)
followed by  y = Re(Winv @ s) @ C + x @ D + Do.

Since d == 1 the whole module is a causal LTI SIMO filter:
    y[t, b, m] = sum_{j>=0} H[j, m] * x[t - j, b] + Do[m]
with impulse response (computed on host in float64)
    H[j, m] = sum_n Re(lam_n^j) * A_re[n, m] - Im(lam_n^j) * A_im[n, m]
    A_re = Re(Winv)^T @ C,  A_im = Im(Winv)^T @ C,  H[0] += D.
A window of 384 lags truncates at 3.7e-3 of max|y| (tolerance is 2e-2).

Precision split (measured exactly on the reference data): head lags
0..127 in f16 (x-quant error ~8x below bf16 at the same 1 cycle/row PE
rate); tail lags 128..383 in ONE fp8e4m3 DoubleRow matmul per chunk
(256-deep contraction, 0.5 cycles/row).  Output is int8 with a single
static scale folded into H on the host: s = 1.05 * max|y| where max|y|
is computed exactly on the host via FFT convolution (x is known), so
PSUM holds y * 126.49/s in [-121, 121] -- no clipping possible -- and
eviction is a plain f32->int8 copy with +0.5 rounding bias.  The host
multiplies the int8 result back by s/126.49.  Measured end-to-end
rel-err 6.5e-3 vs the 2e-2 budget; int8 also halves the output DMA
(4.2 MB/core vs 8.4 MB f16), putting HBM traffic (~7.5 MB/core) and
the PE stream (~21 us) in balance.

Device work (per core, data-parallel over batch: 4 batches/core): for
output chunk t0=128*tci, the f16 stationary operand is Toeplitz slice
    X_i[p, it] = xpad[128*i + p + it]   (i = tci; xpad = 127 zeros ++ x)
against the row-flipped head H tile; the DR stationary is the fp8 copy
of slices (tci-2, tci-1) as a (128, 2, 128) k-tile pair against the
interleaved fp8 tail H (128, 2, 512), accumulated in half of a 2-bank
PSUM tile.  Slices are pre-built on the host (sliding_window_view ->
contiguous DMAs, first-use order round-robined over the 3 queues).
Warm-up matmuls bring the PE HAM clock gate toward 2.4 GHz while
inputs stream in.  PSUM eviction alternates DVE / ACT copies
(f32->int8, +0.5 bias); four 128-row chunks coalesce per output DMA,
all on the sync-engine HWDGE queue (ACT keeps its cycles for
evictions).  Do is handled on the host (zero for this problem).
"""

import os
import sys

sys.path.insert(0, "/opt/trn_rl_repo")

import numpy as np
import ml_dtypes

# problem dims (hardcoded per harness contract)
B, T, N, M = 32, 2048, 512, 512
NCORES = 8
BLOC = B // NCORES          # batches per core
TCH = T // 128              # 128-row output chunks per batch
NLAG = 3                    # 384-lag window: f16 head + fp8 DR tail pair
MODE = os.environ.get("K_MODE", "dr")
QBIAS = float(os.environ.get("K_QBIAS", "0.0"))   # int8 rounding bias (HW rounds to nearest)
SAFETY = 1.05               # static output scale = SAFETY * max|y|


def build_program(t=T, m=M, nlag=NLAG, bloc=BLOC):
    """Build + compile the (SPMD, per-core) Bass program."""
    import concourse.tile as tile
    from concourse import bacc, mybir
    from bass_rust import VecI64Pair

    tch = t // 128
    f32 = mybir.dt.float32
    f16 = mybir.dt.float16
    i8 = mybir.dt.int8
    fp8 = mybir.dt.float8e4
    DR = mybir.MatmulPerfMode.DoubleRow
    Copy = mybir.ActivationFunctionType.Copy

    nc = bacc.Bacc("TRN2", target_bir_lowering=False, debug=False)
    # f16 Toeplitz slices, layout [i][p][b][uu] (one SBUF tile per slice)
    xsh_t = nc.dram_tensor("xsh", [tch * 128, bloc * 128], f16, kind="ExternalInput")
    # fp8 copy, layout [p][b][i][uu] (consecutive i contiguous per batch so a
    # (tci-2, tci-1) pair is one 3D DoubleRow weight AP)
    xf8_t = nc.dram_tensor("xf8", [128, bloc * tch * 128], fp8, kind="ExternalInput")
    # flipped H tiles: head + lag-1 in f16 (stacked), interleaved DR tail fp8
    ht_t = nc.dram_tensor("ht", [2 * 128, m], f16, kind="ExternalInput")
    htdr_t = nc.dram_tensor("htdr", [128, 2 * m], fp8, kind="ExternalInput")
    yq_t = nc.dram_tensor("yq", [bloc, t, m], i8, kind="ExternalOutput")

    nwarm = 9       # PE warm-ups bridge the start barrier -> first input DMA
                    # and keep HAM duty high so full clock lands with the data

    with tile.TileContext(nc) as tc:
        with (
            tc.tile_pool(name="xsh", bufs=1) as xsh_pool,
            tc.tile_pool(name="w", bufs=1) as wpool,
            tc.tile_pool(name="psum", bufs=3, space="PSUM") as psum_pool,
            tc.tile_pool(name="wps", bufs=1, space="PSUM") as warm_pool,
            tc.tile_pool(name="out", bufs=6) as out_pool,
        ):
            # PE warm-up: dummy matmuls on a zeroed tile keep the PE busy
            # through the HAM activity window so real matmuls start fast
            dumb = wpool.tile([128, 256], f16, tag="warm")
            nc.vector.memset(dumb[:], 0.0)
            wps = warm_pool.tile([128, 256], f32)
            for _ in range(nwarm):
                nc.tensor.matmul(
                    wps[:], lhsT=dumb[:, 0:128], rhs=dumb[:],
                    start=True, stop=True
                )

            # persistent weight tiles
            ht0 = wpool.tile([128, m], f16, tag="ht0")
            ht1 = wpool.tile([128, m], f16, tag="ht1")
            htdr = wpool.tile([128, 2 * m], fp8, tag="htdr")
            xf8 = xsh_pool.tile([128, bloc * tch * 128], fp8, tag="xf8")
            xf8_v = xf8[:].rearrange("p (b i uu) -> p b i uu", b=bloc, i=tch)

            # input loads on the three HWDGE queues only (gpsimd SWDGE
            # takes ~1.1us per trigger and starved the PE).  Slices 0-3 +
            # H tiles load as singles for fast first-use; slices 4-15
            # coalesce into three 4-slice DMAs on the vector queue (DVE
            # evictions start late enough to absorb the trigger cost).
            # slice-group-outer chunk order: group g consumes slices
            # 4g..4g+3 (all batches) over ~16 chunk times.
            # tci-major chunk order means slice s is first needed at
            # ~4-chunk cadence -- singles on two HWDGE queues stay ahead;
            # gpsimd (slow SWDGE gen) gets only late-need loads.
            # Queue plan from measured rates (sync ~230GB/s, scalar ~110,
            # gpsimd SWDGE ~1.2us gen per load): sync takes ht tiles + odd
            # slices, scalar only the t0-critical loads, gpsimd everything
            # needed after ~14us.  tci-major demand = one slice per ~2us.
            xsh_sb = [None] * tch
            loads = [(nc.sync, "xsh", 0), (nc.scalar, "ht0", 0),
                     (nc.gpsimd, "f8", 0), (nc.sync, "xsh", 1),
                     (nc.scalar, "ht1", 0), (nc.sync, "xsh", 2),
                     (nc.scalar, "htdr", 0), (nc.gpsimd, "f8", 1),
                     (nc.sync, "xsh", 3), (nc.sync, "xsh", 4),
                     (nc.sync, "xsh", 5), (nc.sync, "xsh", 6),
                     (nc.sync, "xsh", 7), (nc.gpsimd, "xsh", 8),
                     (nc.gpsimd, "xsh", 9), (nc.gpsimd, "xsh", 10),
                     (nc.gpsimd, "xsh", 11), (nc.gpsimd, "f8", 2),
                     (nc.gpsimd, "xsh", 12), (nc.gpsimd, "xsh", 13),
                     (nc.gpsimd, "xsh", 14), (nc.gpsimd, "xsh", 15),
                     (nc.gpsimd, "f8", 3)]
            for eng, kind, i in loads:
                if kind == "ht0":
                    eng.dma_start(ht0[:], ht_t.ap()[0:128, :])
                elif kind == "ht1":
                    eng.dma_start(ht1[:], ht_t.ap()[128:256, :])
                elif kind == "htdr":
                    eng.dma_start(htdr[:], htdr_t.ap())
                elif kind == "f8":
                    in_ap = xf8_t.ap().copy()
                    in_ap.ap = VecI64Pair(
                        [[bloc * tch * 128, 128], [tch * 128, bloc], [1, 4 * 128]]
                    )
                    in_ap.offset = i * 4 * 128
                    eng.dma_start(xf8_v[:, :, 4 * i : 4 * i + 4, :], in_ap)
                else:
                    tl = xsh_pool.tile([128, bloc * 128], f16, tag=f"xsh{i}")
                    eng.dma_start(tl[:], xsh_t.ap()[i * 128 : (i + 1) * 128, :])
                    xsh_sb[i] = tl[:].rearrange("p (b uu) -> p b uu", b=bloc)

            htdr_v = htdr[:].rearrange("p (i n) -> p i n", i=2)

            gi = 0          # eviction-pair index, for engine rotation
            # tci-major: one output tile per slice row = chunks (tci, b0..b3);
            # DRAM write scatters over batches with stride t*m.
            for tci in range(tch):
                last_row = tci == tch - 1
                ot = out_pool.tile([128, bloc * m], i8)
                for pair in range(bloc // 2):
                    ps = psum_pool.tile([128, 2 * m], f32)
                    for half in range(2):
                        b = pair * 2 + half
                        pdst = ps[:, half * m : (half + 1) * m]
                        # head (lags 0..127), f16
                        nc.tensor.matmul(
                            pdst,
                            lhsT=xsh_sb[tci][:, b, :],
                            rhs=ht0[:],
                            start=True,
                            stop=(tci == 0),
                        )
                        if tci == 1:
                            # only one valid tail tile: f16 lag-1 matmul
                            nc.tensor.matmul(
                                pdst,
                                lhsT=xsh_sb[0][:, b, :],
                                rhs=ht1[:],
                                start=False,
                                stop=True,
                            )
                        elif tci >= 2:
                            # lags 128..383 in one fp8 DoubleRow matmul:
                            # k-tile 0 = slice tci-2 (lags 256..383),
                            # k-tile 1 = slice tci-1 (lags 128..255)
                            nc.tensor.matmul(
                                pdst,
                                lhsT=xf8_v[:, b, tci - 2 : tci, :],
                                rhs=htdr_v,
                                start=False,
                                stop=True,
                                perf_mode=DR,
                            )
                    # evict f32 -> int8 (HW rounds to nearest); PSUM
                    # already holds y*126.49/s in [-121, 121]
                    dst = ot[:, pair * 2 * m : (pair + 1) * 2 * m]
                    if last_row:
                        # final row: evict each bank on its own engine
                        # in parallel to shorten the kernel tail
                        nc.vector.tensor_scalar_add(dst[:, :m], ps[:, :m], QBIAS)
                        nc.scalar.activation(dst[:, m:], ps[:, m:], Copy, bias=QBIAS)
                    elif gi % 2 == 0:
                        nc.vector.tensor_scalar_add(dst, ps[:], QBIAS)
                    else:
                        nc.scalar.activation(dst, ps[:], Copy, bias=QBIAS)
                    gi += 1
                otv = ot[:].rearrange("p (c mm) -> p c mm", c=bloc)
                if last_row:
                    # final row: two half-size output DMAs so the first
                    # half streams while the last pair drains
                    for hh in range(2):
                        out_ap = yq_t.ap().copy()
                        out_ap.ap = VecI64Pair([[m, 128], [t * m, 2], [1, m]])
                        out_ap.offset = 2 * hh * t * m + tci * 128 * m
                        [nc.sync, nc.scalar][hh].dma_start(
                            out_ap, otv[:, 2 * hh : 2 * hh + 2, :]
                        )
                else:
                    out_ap = yq_t.ap().copy()
                    out_ap.ap = VecI64Pair([[m, 128], [t * m, bloc], [1, m]])
                    out_ap.offset = tci * 128 * m
                    nc.sync.dma_start(out_ap, otv)

    nc.compile()
    return nc


def host_weights(lnl_re, lnl_im, W_r, W_i, C, D, Do, t=T, m=M, nlag=NLAG,
                 mode=MODE, x=None):
    """Impulse response H (flipped per 128-tile), float64 math.

    The int8 output scale s is folded into H: s = SAFETY * max|y| with
    max|y| computed exactly via FFT convolution when x is given (falls
    back to an 8-sigma statistical bound otherwise).  Returns the input
    map plus "_mult" = s/126.49, the host-side dequantization factor.
    """
    lnl = lnl_re.astype(np.float64) + 1j * lnl_im.astype(np.float64)
    W = W_r.astype(np.float64) + 1j * W_i.astype(np.float64)
    Winv = np.linalg.inv(W)
    A_re = np.ascontiguousarray(Winv.real.T) @ C.astype(np.float64)
    A_im = np.ascontiguousarray(Winv.imag.T) @ C.astype(np.float64)
    j = np.arange(nlag * 128, dtype=np.float64)
    P = np.exp(np.outer(j, lnl))                      # lam^j, (J, N) complex128
    H = P.real @ A_re - P.imag @ A_im                 # (J, M)
    H[0] += D[0].astype(np.float64)

    if x is not None:
        # exact max|y| of the windowed response via FFT linear convolution
        L = 4096                                      # >= T + nlag*128 - 1
        Xf = np.fft.rfft(np.asarray(x, np.float64)[:, :, 0], n=L, axis=1)
        ymax = 0.0
        for mc in range(0, m, 64):
            Hf = np.fft.rfft(H[:, mc : mc + 64], n=L, axis=0)
            Y = np.fft.irfft(Xf[:, :, None] * Hf[None, :, :], n=L, axis=1)
            ymax = max(ymax, np.abs(Y[:, :t, :]).max())
        s = SAFETY * ymax
    else:
        # x ~ N(0,1) exactly, y linear in x: sigma_m^2 = sum_j H[j,m]^2
        s = 8.0 * np.sqrt((H * H).sum(axis=0)).max()

    H *= 126.49 / s
    Hf_t = H.reshape(nlag, 128, m)[:, ::-1, :]        # flipped tiles
    # f16: head tile + lag-1 tile, stacked [2*128, m]
    ht = np.ascontiguousarray(Hf_t[:2].reshape(2 * 128, m)).astype(np.float16)
    # fp8 DR tail: [p, (i, n)] with k-tile 0 = flipped H[256:384], 1 = H[128:256]
    htdr = np.stack([Hf_t[2], Hf_t[1]], axis=1)       # (128, 2, m)
    htdr = np.ascontiguousarray(htdr.reshape(128, 2 * m)).astype(ml_dtypes.float8_e4m3)
    return {"ht": ht, "htdr": htdr, "_mult": float(s / 126.49)}


def make_in_maps(x, weights, t=T, nlag=NLAG, bloc=BLOC, ncores=NCORES, mode=MODE):
    from numpy.lib.stride_tricks import sliding_window_view

    tch = t // 128
    xb = x[:, :, 0].astype(np.float16)                # quantize once, (B, T)
    base = {k: v for k, v in weights.items() if not k.startswith("_")}
    in_maps = []
    for c in range(ncores):
        xpad = np.zeros((127 + t + 1, bloc), np.float16)
        xpad[127 : 127 + t, :] = xb[c * bloc : (c + 1) * bloc].T
        # slice i: X_i[p, b, uu] = xpad[128*i + p + uu, b]
        sw = sliding_window_view(xpad, 128, axis=0)   # sw[k, b, uu] = xpad[k+uu, b]
        xsh = sw[:t].reshape(tch, 128, bloc, 128)     # [i][p][b][uu]
        im = dict(base)
        im["xsh"] = np.ascontiguousarray(xsh).reshape(tch * 128, bloc * 128)
        xf8 = xsh.transpose(1, 2, 0, 3)               # [p][b][i][uu]
        im["xf8"] = np.ascontiguousarray(xf8).reshape(
            128, bloc * tch * 128
        ).astype(ml_dtypes.float8_e4m3)
        in_maps.append(im)
    return in_maps


_prog_cache = {}


def kernel(x, lnl_re, lnl_im, W_r, W_i, C, D, Do):
    from concourse.bass_utils import run_bass_kernel_spmd

    x = np.asarray(x)
    lnl_re, lnl_im = np.asarray(lnl_re), np.asarray(lnl_im)
    W_r, W_i = np.asarray(W_r), np.asarray(W_i)
    C, D, Do = np.asarray(C), np.asarray(D), np.asarray(Do)

    key = (NLAG, MODE)
    if key not in _prog_cache:
        _prog_cache[key] = build_program()
    nc = _prog_cache[key]

    weights = host_weights(lnl_re, lnl_im, W_r, W_i, C, D, Do, x=x)
    in_maps = make_in_maps(np.asarray(x, np.float32), weights)
    res = run_bass_kernel_spmd(nc, in_maps, core_ids=list(range(NCORES)))
    q = np.concatenate(
        [np.asarray(res.results[i]["yq"]) for i in range(NCORES)], axis=0
    )
    y = q.astype(np.float32)
    y *= np.float32(weights["_mult"])
    if np.any(Do):
        y += Do.astype(np.float32)
    return y
